# revision 1
# baseline (speedup 1.0000x reference)
"""Trainium2 Bass kernel for an autoregressive LSTMCell decoder with softmax feedback.

Math (per timestep, PyTorch gate order i,f,g,o):
    gates = [x_t, y] @ W_ih.T + b_ih + h @ W_hh.T + b_hh
    i,f,o = sigmoid(...), g = tanh(...)
    c = f*c + i*g ; h = o*tanh(c) ; y = softmax(h @ W_lin.T + b_lin)

Strategy (8 NeuronCores, data parallel over batch, 32 sequences/core):
  * Features-on-partitions layout: gates.T [2048, 32] packed into one PSUM
    bank [128, (q,j,b)=512]; cell/hidden state packed [128, (k,b)=128].
  * x @ W_x.T precomputed per 32-step chunk as a full-utilization f32r
    matmul (bias folded in via a ones-row); result kept in SBUF as fp16.
  * Recurrent matmuls use fp16 weights as the PE-stationary operand (FWL).
  * All gate activations via one Tanh op (sigmoid(x)=0.5+0.5*tanh(x/2),
    scales folded into weights host-side); softmax via sigma/(1-sigma)
    so only the Sigmoid/Tanh ACT table set is ever used.
  * Cell update carries S=2c, hidden carried as H=2h (absorbed into
    0.5-scaled W_hh / W_lin), giving a 3-op DVE cell update.
"""

import sys

sys.path.insert(0, "/opt/trn_rl_repo")

import numpy as np

import concourse.bass as bass  # noqa: F401
import concourse.tile as tile
from concourse import bacc, mybir
from concourse.bass_utils import run_bass_kernel_spmd
from concourse.masks import make_identity

f32 = mybir.dt.float32
f32r = mybir.dt.float32r
f16 = mybir.dt.float16
AF = mybir.ActivationFunctionType
ALU = mybir.AluOpType

B, D, N = 256, 512, 64
NCORES = 8
BL = B // NCORES  # 32 sequences per core
TC = 32           # timesteps per chunk
P = 128

_CACHE = {}


def _build(T, reps=1):
    NCH = T // TC
    nc = bacc.Bacc("TRN2", target_bir_lowering=False, debug=False, num_devices=NCORES)

    x_d = nc.dram_tensor("x", [NCH, P, 4 * BL * TC], f32, kind="ExternalInput").ap()
    h0_d = nc.dram_tensor("h0", [BL, D], f32, kind="ExternalInput").ap()
    wxt_d = nc.dram_tensor("wxt", [P, 4 * 16 * P], f32, kind="ExternalInput").ap()
    wht_d = nc.dram_tensor("wht", [P, 4 * 16 * P], f16, kind="ExternalInput").ap()
    wyt_d = nc.dram_tensor("wyt", [N, 16 * P], f16, kind="ExternalInput").ap()
    wlt_d = nc.dram_tensor("wlt", [P, 4 * N], f16, kind="ExternalInput").ap()
    bg_d = nc.dram_tensor("bg", [1, 4 * D], f32, kind="ExternalInput").ap()
    bl_d = nc.dram_tensor("bl", [1, N], f16, kind="ExternalInput").ap()
    ones_d = nc.dram_tensor("ones512", [1, 512], f32, kind="ExternalInput").ap()
    out_d = nc.dram_tensor("out", [BL, T, N], f32, kind="ExternalOutput").ap()

    with tile.TileContext(nc) as tc:
        with (
            tc.tile_pool(name="const", bufs=1) as const,
            tc.tile_pool(name="xst", bufs=2) as xst,
            tc.tile_pool(name="gxp", bufs=2) as gxp,
            tc.tile_pool(name="yout", bufs=2) as yout,
            tc.tile_pool(name="state", bufs=2) as state,
            tc.tile_pool(name="tmp", bufs=3) as tmp,
            tc.tile_pool(name="g_ps", bufs=2, space="PSUM") as g_psp,
            tc.tile_pool(name="pre_ps", bufs=2, space="PSUM") as pre_psp,
            tc.tile_pool(name="l_ps", bufs=1, space="PSUM") as l_psp,
            tc.tile_pool(name="tr_ps", bufs=1, space="PSUM") as tr_psp,
        ):
            # ---- constants / weights ----
            wxt = const.tile([P, 4 * 16 * P], f32r)
            nc.sync.dma_start(out=wxt, in_=wxt_d.bitcast(f32r))
            wht = const.tile([P, 4 * 16 * P], f16)
            nc.sync.dma_start(out=wht, in_=wht_d)
            wyt = const.tile([N, 16 * P], f16)
            nc.sync.dma_start(out=wyt, in_=wyt_d)
            wlt = const.tile([P, 4 * N], f16)
            nc.sync.dma_start(out=wlt, in_=wlt_d)
            bg = const.tile([1, 4 * D], f32r)
            nc.sync.dma_start(out=bg, in_=bg_d.bitcast(f32r))
            bl = const.tile([1, N], f16)
            nc.sync.dma_start(out=bl, in_=bl_d)

            ones512 = const.tile([1, 512], f32r)
            nc.sync.dma_start(out=ones512, in_=ones_d.bitcast(f32r))
            ones32 = const.tile([1, BL], f16)
            nc.vector.memset(ones32, 1.0)
            idf = const.tile([P, P], f32)
            make_identity(nc, idf)
            id128 = const.tile([P, P], f16)
            nc.vector.tensor_copy(id128, idf)
            id32 = const.tile([32, 32], f32)
            make_identity(nc, id32)

            for _rep in range(reps):
                # ---- initial state ----
                h0s = const.tile([P, 4, BL], f32)
                h0r = h0_d.rearrange("b (k p) -> k p b", p=P)
                for k in range(4):
                    nc.sync.dma_start(out=h0s[:, k, :], in_=h0r[k])
                H = state.tile([P, 4 * BL], f16, tag="H")
                nc.vector.tensor_scalar(out=H, in0=h0s.rearrange("p k b -> p (k b)"),
                                        scalar1=2.0, scalar2=None, op0=ALU.mult)
                S = state.tile([P, 4 * BL], f32, tag="S")
                nc.vector.memset(S, 0.0)
                yT = state.tile([N, BL], f16, tag="yT")
                nc.vector.memset(yT, 0.0)

                def stage_x(ch):
                    xT = xst.tile([P, 4 * BL * TC], f32r, tag="xT")
                    nc.sync.dma_start(out=xT, in_=x_d[ch].bitcast(f32r))
                    return xT.rearrange("p (k bt) -> p k bt", k=4)

                def pre_mms(xTv, m, half):
                    # one PSUM tile of (sg*W_x) @ x + sg*b for gate-chunk m,
                    # (b,t)-half `half`
                    pp = pre_psp.tile([P, 512], f32, tag="pre")
                    for k in range(4):
                        nc.tensor.matmul(
                            pp,
                            wxt[:, (k * 16 + m) * P:(k * 16 + m + 1) * P],
                            xTv[:, k, half * 512:(half + 1) * 512],
                            start=(k == 0), stop=False, skip_group_check=True,
                        )
                    nc.tensor.matmul(pp, bg[:, m * P:(m + 1) * P], ones512,
                                     start=False, stop=True, skip_group_check=True)
                    return pp

                def pre_copy(pp, GXv, m, half):
                    # permuted copy into GX as fp16
                    ppv = pp.rearrange("c (b t) -> c t b", t=TC)
                    nc.vector.tensor_copy(
                        out=GXv[:, :, 32 * m + 16 * half: 32 * m + 16 * half + 16],
                        in_=ppv)

                def pre_group(xTv, GXv, m, half):
                    pre_copy(pre_mms(xTv, m, half), GXv, m, half)

                prev = None  # pending y-feedback transpose: (Y_tile, tt)
                xTv_next = stage_x(0)
                GX = gxp.tile([P, TC * 512], f16, tag="GX")
                GXv = GX.rearrange("p (t mm) -> p t mm", mm=512)
                for m in range(16):
                    for half in range(2):
                        pre_group(xTv_next, GXv, m, half)
                xTv_cur = xTv_next

                for ch in range(NCH):
                    GX_cur = GX
                    if ch + 1 < NCH:
                        xTv_next = stage_x(ch + 1)
                        GX = gxp.tile([P, TC * 512], f16, tag="GX")
                        GXv = GX.rearrange("p (t mm) -> p t mm", mm=512)
                    Y = yout.tile([BL, TC * N], f32, tag="Y")

                    for tt in range(TC):
                        # --- PE: gx init + W_hh (needs H from prev step) ---
                        g_ps = g_psp.tile([P, 512], f32, tag="g")
                        nc.tensor.matmul(g_ps, id128, GX_cur[:, tt * 512:(tt + 1) * 512],
                                         start=True, stop=False, skip_group_check=True)
                        for m in range(16):
                            for k in range(4):
                                nc.tensor.matmul(
                                    g_ps[:, 32 * m:32 * m + 32],
                                    wht[:, (k * 16 + m) * P:(k * 16 + m + 1) * P],
                                    H[:, 32 * k:32 * k + 32],
                                    start=False, stop=False, skip_group_check=True,
                                )
                        # --- PE: W_y (needs yT) ---
                        for m in range(16):
                            nc.tensor.matmul(
                                g_ps[:, 32 * m:32 * m + 32],
                                wyt[:, m * P:(m + 1) * P],
                                yT,
                                start=False, stop=(m == 15), skip_group_check=True,
                            )
                        # --- PE filler during the tail: next chunk's precompute MMs ---
                        pp_fill = None
                        if ch + 1 < NCH:
                            pp_fill = pre_mms(xTv_next, tt // 2, tt % 2)
                        # --- ACT/DVE tail -> H ---
                        Tg = tmp.tile([P, 512], f32, tag="Tg")
                        nc.scalar.activation(out=Tg[:, 0:384], in_=g_ps[:, 0:384],
                                             func=AF.Tanh, scale=1.0)
                        nc.scalar.activation(out=Tg[:, 384:512], in_=g_ps[:, 384:512],
                                             func=AF.Tanh, scale=1.0)
                        u = tmp.tile([P, 4 * BL], f32, tag="u")
                        nc.vector.scalar_tensor_tensor(out=u, in0=Tg[:, 128:256], scalar=1.0,
                                                       in1=S, op0=ALU.add, op1=ALU.mult)
                        v = tmp.tile([P, 4 * BL], f32, tag="v")
                        nc.vector.scalar_tensor_tensor(out=v, in0=Tg[:, 0:128], scalar=1.0,
                                                       in1=Tg[:, 256:384], op0=ALU.add, op1=ALU.mult)
                        S = state.tile([P, 4 * BL], f32, tag="S")
                        nc.vector.scalar_tensor_tensor(out=S, in0=u, scalar=0.5,
                                                       in1=v, op0=ALU.mult, op1=ALU.add)
                        Tc_ = tmp.tile([P, 4 * BL], f32, tag="Tc")
                        nc.scalar.activation(out=Tc_, in_=S, func=AF.Tanh, scale=0.5)
                        H = state.tile([P, 4 * BL], f16, tag="H")
                        nc.vector.scalar_tensor_tensor(out=H, in0=Tg[:, 384:512], scalar=1.0,
                                                       in1=Tc_, op0=ALU.add, op1=ALU.mult)
                        # --- PE: logits ; ACT/DVE: softmax via sigma/(1-sigma) ---
                        l_ps = l_psp.tile([BL, N], f32, tag="l")
                        for k in range(4):
                            nc.tensor.matmul(l_ps, H[:, 32 * k:32 * k + 32],
                                             wlt[:, N * k:N * (k + 1)],
                                             start=(k == 0), stop=False, skip_group_check=True)
                        nc.tensor.matmul(l_ps, ones32, bl, start=False, stop=True,
                                         skip_group_check=True)
                        e = tmp.tile([BL, N], f32, tag="e")
                        z = tmp.tile([BL, 1], f32, tag="z")
                        nc.scalar.activation(out=e, in_=l_ps, func=AF.Exp, scale=1.0,
                                             accum_out=z)
                        rz = tmp.tile([BL, 1], f32, tag="rz")
                        nc.vector.reciprocal(rz, z)
                        # y feedback first (on the recurrence critical path):
                        # y in f16, then two 32x32 DVE block transposes
                        y16 = tmp.tile([BL, N], f16, tag="y16")
                        nc.vector.tensor_scalar(out=y16, in0=e, scalar1=rz, scalar2=None,
                                                op0=ALU.mult)
                        yT = state.tile([N, BL], f16, tag="yT")
                        nc.vector.transpose(out=yT[0:32, :], in_=y16[:, 0:32])
                        nc.vector.transpose(out=yT[32:64, :], in_=y16[:, 32:64])
                        # y output write (off the critical path)
                        ysl = Y[:, tt * N:(tt + 1) * N]
                        nc.vector.tensor_scalar(out=ysl, in0=e, scalar1=rz, scalar2=None,
                                                op0=ALU.mult)
                        if pp_fill is not None:
                            pre_copy(pp_fill, GXv, tt // 2, tt % 2)
                        prev = (Y, tt)
                    xTv_cur = xTv_next

                    # ---- flush Y chunk ----
                    nc.sync.dma_start(
                        out=out_d[:, ch * TC:(ch + 1) * TC, :].rearrange("b t n -> b (t n)"),
                        in_=Y,
                    )

    nc.compile()
    return nc


def _prep(W_ih, b_ih, W_hh, b_hh, W_lin, b_lin):
    sg = np.concatenate([
        np.full(D, 0.5), np.full(D, 0.5), np.ones(D), np.full(D, 0.5)
    ]).astype(np.float32)
    W_x = (W_ih[:, :D] * sg[:, None]).astype(np.float32)
    W_y = (W_ih[:, D:] * sg[:, None]).astype(np.float16)
    W_h2 = (W_hh * sg[:, None] * 0.5).astype(np.float16)
    b_g = ((b_ih + b_hh) * sg).astype(np.float32)
    W_l2 = (W_lin * 0.5).astype(np.float16)

    wxt = W_x.reshape(16, P, 4, P).transpose(3, 2, 0, 1).reshape(P, 4 * 16 * P).copy()
    wht = W_h2.reshape(16, P, 4, P).transpose(3, 2, 0, 1).reshape(P, 4 * 16 * P).copy()
    wyt = W_y.reshape(16, P, N).transpose(2, 0, 1).reshape(N, 16 * P).copy()
    wlt = W_l2.reshape(N, 4, P).transpose(2, 1, 0).reshape(P, 4 * N).copy()
    return dict(
        wxt=wxt, wht=wht, wyt=wyt, wlt=wlt,
        bg=b_g.reshape(1, 4 * D).copy(),
        bl=b_lin.astype(np.float16).reshape(1, N).copy(),
        ones512=np.ones((1, 512), np.float32),
    )


def make_in_maps(x, init_h, W_ih, b_ih, W_hh, b_hh, W_lin, b_lin):
    x = np.asarray(x, dtype=np.float32)
    T = x.shape[1]
    assert x.shape == (B, T, D) and T % TC == 0
    shared = _prep(np.asarray(W_ih, np.float32), np.asarray(b_ih, np.float32),
                   np.asarray(W_hh, np.float32), np.asarray(b_hh, np.float32),
                   np.asarray(W_lin, np.float32), np.asarray(b_lin, np.float32))
    init_h = np.ascontiguousarray(np.asarray(init_h, np.float32))

    in_maps = []
    for i in range(NCORES):
        m = dict(shared)
        xc = x[i * BL:(i + 1) * BL]  # [BL, T, D]
        xc = xc.reshape(BL, T // TC, TC, 4, P).transpose(1, 4, 3, 0, 2)
        m["x"] = np.ascontiguousarray(xc).reshape(T // TC, P, 4 * BL * TC)
        m["h0"] = np.ascontiguousarray(init_h[i * BL:(i + 1) * BL])
        in_maps.append(m)
    return in_maps, T


def kernel(x, init_h, W_ih, b_ih, W_hh, b_hh, W_lin, b_lin, _trace=False):
    in_maps, T = make_in_maps(x, init_h, W_ih, b_ih, W_hh, b_hh, W_lin, b_lin)
    if T not in _CACHE:
        _CACHE[T] = _build(T)
    nc = _CACHE[T]

    res = run_bass_kernel_spmd(nc, in_maps, list(range(NCORES)), trace=_trace)
    out = np.concatenate([res.results[i]["out"] for i in range(NCORES)], axis=0)
    if _trace:
        kernel.last_exec_time_ns = res.exec_time_ns
        kernel.last_results = res
    return out



# revision 2
# speedup vs baseline: 1.9599x; 1.9599x over previous
"""Trainium2 Bass kernel for an autoregressive LSTMCell decoder with softmax feedback.

Math (per timestep, PyTorch gate order i,f,g,o):
    gates = [x_t, y] @ W_ih.T + b_ih + h @ W_hh.T + b_hh
    i,f,o = sigmoid(...), g = tanh(...)
    c = f*c + i*g ; h = o*tanh(c) ; y = softmax(h @ W_lin.T + b_lin)

Strategy (8 NeuronCores, data parallel over batch, 32 sequences/core):
  * Features-on-partitions: gates.T [2048, 32] packed into one PSUM bank
    [128, (q,b)=512]; cell/hidden state packed [128, (k,b)=128].
  * Gate-chunk order q = (k//2)*8 + gate*2 + (k%2) so h-feature chunks
    k in {0,1} land in PSUM cols 0:256 ("A") and k in {2,3} in 256:512
    ("B").  The ACT/DVE tail (tanh -> cell update -> H) runs per half
    and pipelines against the PE: tanh_A runs while the PE still
    accumulates B columns, and the next step's W_hh k=0,1 matmuls start
    as soon as H_A exists.
  * Recurrent weights (W_hh, W_y) are fp8-e4m3 scaled by 64 (fast
    weight load = 4 elem/cycle; x64 keeps values out of the subnormal
    range).  The x64 is undone for free by the gate tanh's scale=1/64.
  * x @ W_x.T precomputed per 32-step chunk as full-utilization f32r
    matmuls; bias is folded in during the PSUM->SBUF staging copy
    (per-partition scalar add), result kept in SBUF as fp16 (x64).
  * All gate activations via Tanh (sigmoid(x)=0.5+0.5*tanh(x/2), scales
    folded into weights host-side); softmax via Exp + reciprocal -- both
    functions live in the single "exp_and_others" ACT table set.
  * Cell state carried as S=2c, hidden as H=2h (absorbed into
    0.5-scaled W_hh / W_lin).  State math in fp16 (2x DVE mode).
  * y written to DRAM in fp16; host converts to f32.
"""

import sys

sys.path.insert(0, "/opt/trn_rl_repo")

import numpy as np
import ml_dtypes

import concourse.bass as bass  # noqa: F401
import concourse.tile as tile
from concourse import bacc, mybir
from concourse.bass_utils import run_bass_kernel_spmd
from concourse.masks import make_identity

f32 = mybir.dt.float32
f32r = mybir.dt.float32r
f16 = mybir.dt.float16
f8 = mybir.dt.float8e4
AF = mybir.ActivationFunctionType
ALU = mybir.AluOpType

B, D, N = 256, 512, 64
NCORES = 8
BL = B // NCORES  # 32 sequences per core
TC = 32           # timesteps per chunk
P = 128
SCALE = 64.0      # fp8 weight scale, undone by tanh ACT scale

# chunk order: old chunk index m = gate*4 + k  ->  position q
# q = (k//2)*8 + gate*2 + (k%2):  A half (cols 0:256) holds k in {0,1}
_PERM = [0] * 16
for _q in range(16):
    _k = (_q // 8) * 2 + (_q % 2)
    _gate = (_q % 8) // 2
    _PERM[_q] = _gate * 4 + _k

_CACHE = {}


def _build(T, reps=1):
    NCH = T // TC
    nc = bacc.Bacc("TRN2", target_bir_lowering=False, debug=False, num_devices=NCORES)

    x_d = nc.dram_tensor("x", [NCH, P, 4 * BL * TC], f32, kind="ExternalInput").ap()
    h0_d = nc.dram_tensor("h0", [BL, D], f32, kind="ExternalInput").ap()
    wxt_d = nc.dram_tensor("wxt", [P, 4 * 16 * P], f32, kind="ExternalInput").ap()
    wht_d = nc.dram_tensor("wht", [P, 4 * 16 * P], f8, kind="ExternalInput").ap()
    wyt_d = nc.dram_tensor("wyt", [N, 16 * P], f8, kind="ExternalInput").ap()
    wlt_d = nc.dram_tensor("wlt", [P, 4 * N], f16, kind="ExternalInput").ap()
    bgt_d = nc.dram_tensor("bgt", [P, 16], f32, kind="ExternalInput").ap()
    bl_d = nc.dram_tensor("bl", [1, N], f16, kind="ExternalInput").ap()
    out_d = nc.dram_tensor("out", [BL, T, N], f16, kind="ExternalOutput").ap()

    with tile.TileContext(nc) as tc:
        with (
            tc.tile_pool(name="const", bufs=1) as const,
            tc.tile_pool(name="xst", bufs=2) as xst,
            tc.tile_pool(name="gxp", bufs=2) as gxp,
            tc.tile_pool(name="yout", bufs=2) as yout,
            tc.tile_pool(name="state", bufs=2) as state,
            tc.tile_pool(name="tmp", bufs=3) as tmp,
            tc.tile_pool(name="g_ps", bufs=2, space="PSUM") as g_psp,
            tc.tile_pool(name="pre_ps", bufs=2, space="PSUM") as pre_psp,
            tc.tile_pool(name="l_ps", bufs=2, space="PSUM") as l_psp,
        ):
            # ---- constants / weights ----
            wxt = const.tile([P, 4 * 16 * P], f32r)
            nc.sync.dma_start(out=wxt, in_=wxt_d.bitcast(f32r))
            wht = const.tile([P, 4 * 16 * P], f8)
            nc.sync.dma_start(out=wht, in_=wht_d)
            wyt = const.tile([N, 16 * P], f8)
            nc.sync.dma_start(out=wyt, in_=wyt_d)
            wlt = const.tile([P, 4 * N], f16)
            nc.sync.dma_start(out=wlt, in_=wlt_d)
            bgt = const.tile([P, 16], f32)
            nc.sync.dma_start(out=bgt, in_=bgt_d)
            bl = const.tile([1, N], f16)
            nc.sync.dma_start(out=bl, in_=bl_d)

            ones32 = const.tile([1, BL], f16)
            nc.vector.memset(ones32, 1.0)
            idf = const.tile([P, P], f32)
            make_identity(nc, idf)
            id8 = const.tile([P, P], f8)
            nc.vector.tensor_copy(id8, idf)

            for _rep in range(reps):
                # ---- initial state ----
                h0s = const.tile([P, 4, BL], f32)
                h0r = h0_d.rearrange("b (k p) -> k p b", p=P)
                for k in range(4):
                    nc.sync.dma_start(out=h0s[:, k, :], in_=h0r[k])
                H = state.tile([P, 4 * BL], f16, tag="H")
                nc.vector.tensor_scalar(out=H, in0=h0s.rearrange("p k b -> p (k b)"),
                                        scalar1=2.0, scalar2=None, op0=ALU.mult)
                S = state.tile([P, 4 * BL], f16, tag="S")
                nc.vector.memset(S, 0.0)
                yT = state.tile([N, BL], f16, tag="yT")
                nc.vector.memset(yT, 0.0)

                def stage_x(ch):
                    xT = xst.tile([P, 4 * BL * TC], f32r, tag="xT")
                    nc.sync.dma_start(out=xT, in_=x_d[ch].bitcast(f32r))
                    return xT.rearrange("p (k bt) -> p k bt", k=4)

                def pre_mms(xTv, q, half):
                    # one PSUM tile of (64*sg*W_x) @ x for gate-chunk q,
                    # batch-half `half` (all 32 timesteps of the chunk)
                    pp = pre_psp.tile([P, 512], f32, tag="pre")
                    for kx in range(4):
                        nc.tensor.matmul(
                            pp,
                            wxt[:, (kx * 16 + q) * P:(kx * 16 + q + 1) * P],
                            xTv[:, kx, half * 512:(half + 1) * 512],
                            start=(kx == 0), stop=(kx == 3), skip_group_check=True,
                        )
                    return pp

                def pre_copy(pp, GXv, q, half):
                    # permuted copy into GX as fp16, bias folded in
                    ppv = pp.rearrange("c (b t) -> c t b", t=TC)
                    nc.vector.tensor_scalar(
                        out=GXv[:, :, 32 * q + 16 * half: 32 * q + 16 * half + 16],
                        in0=ppv, scalar1=bgt[:, q:q + 1], scalar2=None, op0=ALU.add)

                def pre_group(xTv, GXv, q, half):
                    pre_copy(pre_mms(xTv, q, half), GXv, q, half)

                xTv_next = stage_x(0)
                GX = gxp.tile([P, TC * 512], f16, tag="GX")
                GXv = GX.rearrange("p (t mm) -> p t mm", mm=512)
                for q in range(16):
                    for half in range(2):
                        pre_group(xTv_next, GXv, q, half)

                for ch in range(NCH):
                    GX_cur = GX
                    if ch + 1 < NCH:
                        xTv_next = stage_x(ch + 1)
                        GX = gxp.tile([P, TC * 512], f16, tag="GX")
                        GXv = GX.rearrange("p (t mm) -> p t mm", mm=512)
                    Y = yout.tile([BL, TC * N], f16, tag="Y")

                    for tt in range(TC):
                        # ---------- PE: accumulate gates for step tt ----------
                        g_ps = g_psp.tile([P, 512], f32, tag="g")
                        nc.tensor.matmul(g_ps, id8, GX_cur[:, tt * 512:(tt + 1) * 512],
                                         start=True, stop=False, skip_group_check=True)
                        # A half (output chunks q=0..7), contraction k=0,1 first
                        # (needs only H_A of the previous step), then k=2,3.
                        for half_q in range(2):
                            q0 = 8 * half_q
                            for kpair in range(2):
                                for q in range(q0, q0 + 8):
                                    for k in (2 * kpair, 2 * kpair + 1):
                                        nc.tensor.matmul(
                                            g_ps[:, 32 * q:32 * q + 32],
                                            wht[:, (k * 16 + q) * P:(k * 16 + q + 1) * P],
                                            H[:, 32 * k:32 * k + 32],
                                            start=False, stop=False,
                                            skip_group_check=True,
                                        )
                            # y feedback for this half (latest dependency: yT)
                            for q in range(q0, q0 + 8):
                                nc.tensor.matmul(
                                    g_ps[:, 32 * q:32 * q + 32],
                                    wyt[:, q * P:(q + 1) * P],
                                    yT,
                                    start=False, stop=(q == q0 + 7),
                                    skip_group_check=True,
                                )
                            if half_q == 0:
                                # ---- ACT/DVE tail for the A half ----
                                TgA = tmp.tile([P, 256], f16, tag="TgA")
                                nc.scalar.activation(out=TgA, in_=g_ps[:, 0:256],
                                                     func=AF.Tanh, scale=1.0 / SCALE)
                        # ---- PE: logits bias + k=0,1 (H_A-dependent) ----
                        l_ps = l_psp.tile([BL, N], f32, tag="l")
                        nc.tensor.matmul(l_ps, ones32, bl, start=True, stop=False,
                                         skip_group_check=True)
                        # ---- DVE: cell update, A half ----
                        S_new = state.tile([P, 4 * BL], f16, tag="S")
                        H_new = state.tile([P, 4 * BL], f16, tag="H")
                        uA = tmp.tile([P, 2 * BL], f16, tag="uA")
                        nc.vector.scalar_tensor_tensor(
                            out=uA, in0=TgA[:, 64:128], scalar=1.0,
                            in1=S[:, 0:64], op0=ALU.add, op1=ALU.mult)
                        vA = tmp.tile([P, 2 * BL], f16, tag="vA")
                        nc.vector.scalar_tensor_tensor(
                            out=vA, in0=TgA[:, 0:64], scalar=1.0,
                            in1=TgA[:, 128:192], op0=ALU.add, op1=ALU.mult)
                        nc.vector.scalar_tensor_tensor(
                            out=S_new[:, 0:64], in0=uA, scalar=0.5,
                            in1=vA, op0=ALU.mult, op1=ALU.add)
                        TcA = tmp.tile([P, 2 * BL], f16, tag="TcA")
                        nc.scalar.activation(out=TcA, in_=S_new[:, 0:64],
                                             func=AF.Tanh, scale=0.5)
                        nc.vector.scalar_tensor_tensor(
                            out=H_new[:, 0:64], in0=TgA[:, 192:256], scalar=1.0,
                            in1=TcA, op0=ALU.add, op1=ALU.mult)
                        # ---- PE: logits k=0,1 need only H_A ----
                        for k in range(2):
                            nc.tensor.matmul(l_ps, H_new[:, 32 * k:32 * k + 32],
                                             wlt[:, N * k:N * (k + 1)],
                                             start=False, stop=False,
                                             skip_group_check=True)
                        # ---- PE filler: next chunk's precompute MMs ----
                        pp_fill = None
                        if ch + 1 < NCH:
                            pp_fill = pre_mms(xTv_next, tt // 2, tt % 2)
                        # ---- ACT/DVE tail for the B half ----
                        TgB = tmp.tile([P, 256], f16, tag="TgB")
                        nc.scalar.activation(out=TgB, in_=g_ps[:, 256:512],
                                             func=AF.Tanh, scale=1.0 / SCALE)
                        uB = tmp.tile([P, 2 * BL], f16, tag="uB")
                        nc.vector.scalar_tensor_tensor(
                            out=uB, in0=TgB[:, 64:128], scalar=1.0,
                            in1=S[:, 64:128], op0=ALU.add, op1=ALU.mult)
                        vB = tmp.tile([P, 2 * BL], f16, tag="vB")
                        nc.vector.scalar_tensor_tensor(
                            out=vB, in0=TgB[:, 0:64], scalar=1.0,
                            in1=TgB[:, 128:192], op0=ALU.add, op1=ALU.mult)
                        nc.vector.scalar_tensor_tensor(
                            out=S_new[:, 64:128], in0=uB, scalar=0.5,
                            in1=vB, op0=ALU.mult, op1=ALU.add)
                        TcB = tmp.tile([P, 2 * BL], f16, tag="TcB")
                        nc.scalar.activation(out=TcB, in_=S_new[:, 64:128],
                                             func=AF.Tanh, scale=0.5)
                        nc.vector.scalar_tensor_tensor(
                            out=H_new[:, 64:128], in0=TgB[:, 192:256], scalar=1.0,
                            in1=TcB, op0=ALU.add, op1=ALU.mult)
                        # ---- PE: logits k=2,3 (need H_B) ----
                        for k in range(2, 4):
                            nc.tensor.matmul(l_ps, H_new[:, 32 * k:32 * k + 32],
                                             wlt[:, N * k:N * (k + 1)],
                                             start=False, stop=(k == 3),
                                             skip_group_check=True)
                        # ---- ACT/DVE: softmax -> y (fp16) -> yT feedback ----
                        e = tmp.tile([BL, N], f16, tag="e")
                        z = tmp.tile([BL, 1], f32, tag="z")
                        nc.scalar.activation(out=e, in_=l_ps, func=AF.Exp, scale=1.0,
                                             accum_out=z)
                        rz = tmp.tile([BL, 1], f32, tag="rz")
                        nc.vector.reciprocal(rz, z)
                        ysl = Y[:, tt * N:(tt + 1) * N]
                        nc.vector.tensor_scalar(out=ysl, in0=e, scalar1=rz,
                                                scalar2=None, op0=ALU.mult)
                        yT = state.tile([N, BL], f16, tag="yT")
                        nc.vector.transpose(out=yT[0:32, :], in_=ysl[:, 0:32])
                        nc.vector.transpose(out=yT[32:64, :], in_=ysl[:, 32:64])
                        if pp_fill is not None:
                            pre_copy(pp_fill, GXv, tt // 2, tt % 2)
                        S = S_new
                        H = H_new

                    # ---- flush Y chunk ----
                    nc.sync.dma_start(
                        out=out_d[:, ch * TC:(ch + 1) * TC, :].rearrange("b t n -> b (t n)"),
                        in_=Y,
                    )

    nc.compile()
    return nc


def _prep(W_ih, b_ih, W_hh, b_hh, W_lin, b_lin):
    sg = np.concatenate([
        np.full(D, 0.5), np.full(D, 0.5), np.ones(D), np.full(D, 0.5)
    ]).astype(np.float32)
    perm = np.array(_PERM)
    W_x = (W_ih[:, :D] * sg[:, None] * SCALE).astype(np.float32)
    W_y8 = (W_ih[:, D:] * sg[:, None] * SCALE).astype(ml_dtypes.float8_e4m3)
    W_h8 = (W_hh * sg[:, None] * 0.5 * SCALE).astype(ml_dtypes.float8_e4m3)
    b_g = ((b_ih + b_hh) * sg * SCALE).astype(np.float32)
    W_l2 = (W_lin * 0.5).astype(np.float16)

    wxt = (W_x.reshape(16, P, 4, P)[perm]
           .transpose(3, 2, 0, 1).reshape(P, 4 * 16 * P).copy())
    wht = (W_h8.reshape(16, P, 4, P)[perm]
           .transpose(3, 2, 0, 1).reshape(P, 4 * 16 * P).copy())
    wyt = (W_y8.reshape(16, P, N)[perm]
           .transpose(2, 0, 1).reshape(N, 16 * P).copy())
    wlt = W_l2.reshape(N, 4, P).transpose(2, 1, 0).reshape(P, 4 * N).copy()
    bgt = b_g.reshape(16, P)[perm].T.copy()
    return dict(
        wxt=wxt, wht=wht, wyt=wyt, wlt=wlt, bgt=bgt,
        bl=b_lin.astype(np.float16).reshape(1, N).copy(),
    )


def make_in_maps(x, init_h, W_ih, b_ih, W_hh, b_hh, W_lin, b_lin):
    x = np.asarray(x, dtype=np.float32)
    T = x.shape[1]
    assert x.shape == (B, T, D) and T % TC == 0
    shared = _prep(np.asarray(W_ih, np.float32), np.asarray(b_ih, np.float32),
                   np.asarray(W_hh, np.float32), np.asarray(b_hh, np.float32),
                   np.asarray(W_lin, np.float32), np.asarray(b_lin, np.float32))
    init_h = np.ascontiguousarray(np.asarray(init_h, np.float32))

    in_maps = []
    for i in range(NCORES):
        m = dict(shared)
        xc = x[i * BL:(i + 1) * BL]  # [BL, T, D]
        xc = xc.reshape(BL, T // TC, TC, 4, P).transpose(1, 4, 3, 0, 2)
        m["x"] = np.ascontiguousarray(xc).reshape(T // TC, P, 4 * BL * TC)
        m["h0"] = np.ascontiguousarray(init_h[i * BL:(i + 1) * BL])
        in_maps.append(m)
    return in_maps, T


def kernel(x, init_h, W_ih, b_ih, W_hh, b_hh, W_lin, b_lin, _trace=False):
    in_maps, T = make_in_maps(x, init_h, W_ih, b_ih, W_hh, b_hh, W_lin, b_lin)
    if T not in _CACHE:
        _CACHE[T] = _build(T)
    nc = _CACHE[T]

    res = run_bass_kernel_spmd(nc, in_maps, list(range(NCORES)), trace=_trace)
    out = np.concatenate(
        [res.results[i]["out"].astype(np.float32) for i in range(NCORES)], axis=0)
    if _trace:
        kernel.last_exec_time_ns = res.exec_time_ns
        kernel.last_results = res
    return out
